# revision 13
# baseline (speedup 1.0000x reference)
"""MultiHeadDifferentialAttention Trainium2 kernel (8 NeuronCores).

Sharding: core c handles batch b = c // 4 and heads [4*(c%4), 4*(c%4)+4).
Each core computes its 4 heads' differential attention and a partial output
projection y^T = (0.8 * Wp[head_rows])^T @ attT; the host sums the 4 partials
per batch and adds proj_b.

Per-core layout (T=2048, D=64, C=1024, 4 heads = 2 head-pairs):
  - projections: stationary W chunks (c-chunk 128, 128 = two heads' d cols),
    moving x^T chunks -> q1p/q2p/k1p/k2p/v in pair-major (128=[hA d|hB d], T)
    bf16 layout, single-copy PSUM evacuation on VectorE.
  - scores S^T[k, q] via 64-row PE tiling: tile T0 computes head A (rows
    0:64), tile T8 head B, each doing both variants; 4 score regions per
    round are bank-disjoint (T0 -> bank 0, T8 -> bank 1 of each slot).
  - softmax: no max subtraction (|scores| <= ~0.6); one ScalarE Exp per
    2-k-tile round covering 2 heads x 2 variants with scale=1/8 folded in;
    causal diagonal tile masked by a 0/1 triangle multiply on GpSimd.
  - PV: flipped matmul (stationary = exp tile, moving = V_aug (k,65) with a
    ones column) -> att[q, d] q-major with row sums in column 64.
  - combine on VectorE: att = att1/sum1 - lam*att2/sum2 (the 0.8 factor is
    folded into Wp on the host); subtract on GpSimd.
  - output projection: PE-transpose att to hd-major, y^T = Wp^T @ attT,
    pipelined with the second pair's attention (qi-outer ordering).
"""

import numpy as np
import ml_dtypes

import concourse.bass as bass
from concourse import bacc
import concourse.mybir as mybir
import concourse.tile as tile
from concourse.bass_utils import run_bass_kernel_spmd

BF16 = mybir.dt.bfloat16
F32 = mybir.dt.float32

B, T, C = 2, 2048, 1024
H, D = 16, 64
N_CORES = 8
INITIAL_LAMBDA = 0.2  # 0.8 - 0.6*exp(-0.3*(1-1))
OUT_SCALE = 1.0 - INITIAL_LAMBDA  # folded into proj_w on host

NT = T // 128  # 16 k/q tiles
NCC = C // 128  # 8 contraction chunks for projections
K_CHUNK = 2  # score k-tiles per psum round (2 banks per slot, 4 regions)
PROJS = 5  # q1, k1, q2, k2, v


def build_nc(phases=99):
    nc = bacc.Bacc(None)

    xt_d = nc.declare_dram_parameter("xt", [128, NCC, T], BF16, isOutput=False)
    w5_d = nc.declare_dram_parameter("w5", [128, PROJS, 2, NCC, 128], BF16, isOutput=False)
    wp_d = nc.declare_dram_parameter("wp", [128, 2, NCC, 128], BF16, isOutput=False)
    lam_d = nc.declare_dram_parameter("lam", [128, 1], F32, isOutput=False)
    tri_d = nc.declare_dram_parameter("tri", [128, 128], BF16, isOutput=False)
    ident_d = nc.declare_dram_parameter("ident", [128, 128], BF16, isOutput=False)
    yt_d = nc.declare_dram_parameter("yt", [C, T], F32, isOutput=True)

    with tile.TileContext(nc) as tc:
        with (
            tc.tile_pool(name="const", bufs=1) as const,
            tc.tile_pool(name="big", bufs=1) as big,
            tc.tile_pool(name="work", bufs=2) as work,
        ):
            # ---- loads ----
            xt = big.tile([128, NCC, T], BF16, tag="xt")
            for tb in range(4):
                nc.sync.dma_start(out=xt[:, :, tb * 512:(tb + 1) * 512],
                                  in_=xt_d[:, :, tb * 512:(tb + 1) * 512])
            w5 = big.tile([128, PROJS, 2, NCC, 128], BF16, tag="w5")
            for pair in range(2):
                nc.sync.dma_start(out=w5[:, :, pair, :, :], in_=w5_d[:, :, pair, :, :])
            wp = big.tile([128, 2, NCC, 128], BF16, tag="wp")
            nc.sync.dma_start(out=wp[:], in_=wp_d[:])
            lam = const.tile([128, 1], F32, tag="lam")
            nc.sync.dma_start(out=lam[:], in_=lam_d[:])
            tri = const.tile([128, 128], BF16, tag="tri")
            nc.sync.dma_start(out=tri[:], in_=tri_d[:])
            ident = const.tile([128, 128], BF16, tag="ident")
            nc.sync.dma_start(out=ident[:], in_=ident_d[:])

            # persistent tensors: pair-major [hA d 0:64 | hB d 64:128]
            q1p = [big.tile([128, T], BF16, tag=f"q1p{p}", name=f"q1p{p}") for p in range(2)]
            q2p = [big.tile([128, T], BF16, tag=f"q2p{p}", name=f"q2p{p}") for p in range(2)]
            k1p = [big.tile([128, T], BF16, tag=f"k1p{p}", name=f"k1p{p}") for p in range(2)]
            k2p = [big.tile([128, T], BF16, tag=f"k2p{p}", name=f"k2p{p}") for p in range(2)]
            vaug = [big.tile([128, NT, 65], BF16, tag=f"va{h}", name=f"va{h}") for h in range(4)]
            attq = [big.tile([128, 256], BF16, tag=f"at{q}", name=f"at{q}") for q in range(NT)]
            attT = [big.tile([128, T], BF16, tag=f"aT{hc}", name=f"aT{hc}") for hc in range(2)]
            vt_pair = [work.tile([128, T], BF16, tag="vt", name=f"vt{i}") for i in range(2)]

            psB = tc.tile_pool(name="psB", bufs=2, space="PSUM")
            psc_b = psB.__enter__()
            psD = tc.tile_pool(name="psD", bufs=2, space="PSUM")
            psc_d = psD.__enter__()

            PDEST = {0: q1p, 1: k1p, 2: q2p, 3: k2p}

            def emit_pair(pair):
                hA, hB = 2 * pair, 2 * pair + 1
                for proj in range(PROJS):
                    for tb in range(4):
                        ps = psc_b.tile([128, 512], F32, tag="pp", name=f"pp{pair}{proj}{tb}")
                        for cc in range(NCC):
                            nc.tensor.matmul(
                                ps[:],
                                lhsT=w5[:, proj, pair, cc, :],
                                rhs=xt[:, cc, tb * 512:(tb + 1) * 512],
                                start=(cc == 0),
                                stop=(cc == NCC - 1),
                            )
                        ts = slice(tb * 512, (tb + 1) * 512)
                        if proj == 4:
                            nc.vector.tensor_copy(vt_pair[pair][:, ts], ps[:])
                        else:
                            nc.vector.tensor_copy(PDEST[proj][pair][:, ts], ps[:])
                # V transpose (both heads at once)
                for tt in range(NT):
                    pst = psc_b.tile([128, 128], BF16, tag="pp", name=f"pt{pair}{tt}")
                    nc.tensor.transpose(
                        pst[:], vt_pair[pair][:, tt * 128:(tt + 1) * 128], ident[:]
                    )
                    nc.vector.tensor_copy(vaug[hA][:, tt, 0:64], pst[:, 0:64])
                    nc.vector.tensor_copy(vaug[hB][:, tt, 0:64], pst[:, 64:128])
                nc.vector.memset(vaug[hA][:, :, 64:65], 1.0)
                nc.vector.memset(vaug[hB][:, :, 64:65], 1.0)

            def emit_attn(pair, qi):
                # region r = hh*2 + v: r0/r1 = head A v1/v2 (T0, bank 0 of slot),
                # r2/r3 = head B (T8, bank 1)
                hA, hB = 2 * pair, 2 * pair + 1
                qs = slice(qi * 128, (qi + 1) * 128)
                expt = work.tile([128, 4, NT, 128], BF16, tag="expt", name=f"ex{pair}{qi}")
                pa = psc_d.tile([128, 4, 65], F32, tag="pa", name=f"pa{pair}{qi}")
                n_t = qi + 1
                for base in range(0, n_t, K_CHUNK):
                    ln = min(K_CHUNK, n_t - base)
                    ps_s = psc_d.tile([128, 4, K_CHUNK, 128], F32, tag="ps_s",
                                      name=f"ss{pair}{qi}{base}")
                    for j in range(ln):
                        t = base + j
                        ks = slice(t * 128, (t + 1) * 128)
                        for hh, rows in ((0, slice(0, 64)), (1, slice(64, 128))):
                            tp = (hh * 64, 0)
                            nc.tensor.matmul(
                                ps_s[:, hh * 2 + 0, j, :],
                                lhsT=k1p[pair][rows, ks], rhs=q1p[pair][rows, qs],
                                start=True, stop=True, tile_position=tp,
                            )
                            nc.tensor.matmul(
                                ps_s[:, hh * 2 + 1, j, :],
                                lhsT=k2p[pair][rows, ks], rhs=q2p[pair][rows, qs],
                                start=True, stop=True, tile_position=tp,
                            )
                    nc.scalar.activation(
                        expt[:, :, base:base + ln, :],
                        ps_s[:, :, 0:ln, :],
                        mybir.ActivationFunctionType.Exp,
                        scale=0.125,
                    )
                # causal mask on diagonal tile, all 4 regions at once
                nc.gpsimd.tensor_tensor(
                    out=expt[:, :, qi, :], in0=expt[:, :, qi, :],
                    in1=tri[:].unsqueeze(1).broadcast_to([128, 4, 128]),
                    op=mybir.AluOpType.mult,
                )
                # PV: full-contraction, accumulate over t
                for hh in range(2):
                    for v in range(2):
                        r = hh * 2 + v
                        for t in range(n_t):
                            nc.tensor.matmul(
                                pa[:, r, :],
                                lhsT=expt[:, r, t, :],
                                rhs=vaug[2 * pair + hh][:, t, :],
                                start=(t == 0), stop=(t == n_t - 1),
                            )
                # combine
                rc = work.tile([128, 4], F32, tag="rc", name=f"rc{pair}{qi}")
                nc.vector.reciprocal(rc[:], pa[:, :, 64:65].squeeze(2))
                for hh, h in ((0, hA), (1, hB)):
                    o1 = work.tile([128, 64], F32, tag="o1", name=f"o1_{pair}{qi}{hh}")
                    o2 = work.tile([128, 64], F32, tag="o2", name=f"o2_{pair}{qi}{hh}")
                    nc.vector.tensor_scalar_mul(
                        o1[:], pa[:, hh * 2 + 0, 0:64], rc[:, hh * 2:hh * 2 + 1])
                    nc.vector.tensor_scalar(
                        out=o2[:], in0=pa[:, hh * 2 + 1, 0:64],
                        scalar1=rc[:, hh * 2 + 1:hh * 2 + 2], scalar2=lam[:],
                        op0=mybir.AluOpType.mult, op1=mybir.AluOpType.mult,
                    )
                    lh = h % 4
                    nc.gpsimd.tensor_tensor(
                        out=attq[qi][:, lh * 64:(lh + 1) * 64],
                        in0=o1[:], in1=o2[:], op=mybir.AluOpType.subtract,
                    )

            def emit_attT(qi):
                for hc in range(2):
                    pst = psc_b.tile([128, 128], BF16, tag="pp", name=f"pT{qi}{hc}")
                    nc.tensor.transpose(
                        pst[:], attq[qi][:, hc * 128:(hc + 1) * 128], ident[:]
                    )
                    nc.vector.tensor_copy(attT[hc][:, qi * 128:(qi + 1) * 128], pst[:])

            def emit_outproj(tb):
                ts = slice(tb * 512, (tb + 1) * 512)
                for et in range(NCC):
                    py = psc_b.tile([128, 512], F32, tag="pp", name=f"py{et}{tb}")
                    for hc in range(2):
                        nc.tensor.matmul(
                            py[:],
                            lhsT=wp[:, hc, et, :],
                            rhs=attT[hc][:, ts],
                            start=(hc == 0), stop=(hc == 1),
                        )
                    ys = work.tile([128, 512], F32, tag="ys", name=f"ys{et}{tb}")
                    nc.scalar.copy(ys[:], py[:])
                    nc.sync.dma_start(out=yt_d[et * 128:(et + 1) * 128, ts], in_=ys[:])

            emit_pair(0)
            if phases >= 2:
                for qi in range(NT):
                    emit_attn(0, qi)
            emit_pair(1)
            if phases >= 2:
                for qi in range(NT):
                    emit_attn(1, qi)
                    if phases >= 3:
                        emit_attT(qi)
                        if qi % 4 == 3:
                            emit_outproj(qi // 4)

            if phases < 2:
                for et in range(NCC):
                    dbg = work.tile([128, T], F32, tag="ysdbg", name=f"dbg{et}")
                    nc.vector.tensor_copy(dbg[:], q1p[et % 2][:])
                    nc.sync.dma_start(out=yt_d[et * 128:(et + 1) * 128, :], in_=dbg[:])
            if phases == 2:
                for et in range(NCC):
                    dbg = work.tile([128, 256], F32, tag="ysdbg", name=f"dbg{et}")
                    nc.vector.tensor_copy(dbg[:], attq[et][:, :])
                    nc.sync.dma_start(out=yt_d[et * 128:(et + 1) * 128, 0:256], in_=dbg[:])

            psD.__exit__(None, None, None)
            psB.__exit__(None, None, None)

    nc.compile()
    from waitsplit import split_multiwaits
    split_multiwaits(nc, max_waits=1)
    return nc


_NC_CACHE = None


def _get_nc():
    global _NC_CACHE
    if _NC_CACHE is None:
        _NC_CACHE = build_nc()
    return _NC_CACHE


def _prep_in_maps(x, key1, key2, query1, query2, value, lambdas, proj_w):
    bf = ml_dtypes.bfloat16
    lam_val = float(
        np.exp(np.float64(lambdas[0]) * np.float64(lambdas[1]))
        - np.exp(np.float64(lambdas[2]) * np.float64(lambdas[3]))
        + INITIAL_LAMBDA
    )
    lam_t = np.full((128, 1), lam_val, np.float32)
    tri = np.triu(np.ones((128, 128), np.float32)).astype(bf)  # tri[k,j]=1 if j>=k
    ident = np.eye(128, dtype=np.float32).astype(bf)

    wstack = np.stack([query1, key1, query2, key2, value], 0)  # (5, H, C, D)

    in_maps = []
    for c in range(N_CORES):
        b = c // 4
        g = c % 4
        heads = range(4 * g, 4 * g + 4)

        xt = np.ascontiguousarray(
            x[b].T.reshape(NCC, 128, T).transpose(1, 0, 2)).astype(bf)

        w5 = np.empty((128, PROJS, 2, NCC, 128), np.float32)
        for p in range(PROJS):
            for pair in range(2):
                hA = 4 * g + 2 * pair
                wcat = np.concatenate([wstack[p, hA], wstack[p, hA + 1]], axis=1)
                w5[:, p, pair] = wcat.reshape(NCC, 128, 128).transpose(1, 0, 2)
        w5 = w5.astype(bf)

        rows = np.concatenate([np.arange(h * D, (h + 1) * D) for h in heads])
        wp_full = (OUT_SCALE * proj_w[rows]).astype(np.float32)  # (256, C)
        wp = np.empty((128, 2, NCC, 128), np.float32)
        for hc in range(2):
            wp[:, hc] = wp_full[hc * 128:(hc + 1) * 128].reshape(128, NCC, 128)
        wp = wp.astype(bf)

        in_maps.append({
            "xt": xt, "w5": w5, "wp": wp, "lam": lam_t, "tri": tri, "ident": ident,
        })
    return in_maps


def kernel(x, key1, key2, query1, query2, value, lambdas, proj_w, proj_b):
    nc = _get_nc()
    in_maps = _prep_in_maps(
        np.asarray(x, np.float32), np.asarray(key1, np.float32),
        np.asarray(key2, np.float32), np.asarray(query1, np.float32),
        np.asarray(query2, np.float32), np.asarray(value, np.float32),
        np.asarray(lambdas, np.float32), np.asarray(proj_w, np.float32),
    )
    res = run_bass_kernel_spmd(nc, in_maps, list(range(N_CORES)))
    out = np.zeros((B, T, C), np.float32)
    for c in range(N_CORES):
        out[c // 4] += res.results[c]["yt"].T
    out += np.asarray(proj_b, np.float32)[None, None, :]
    return out
